# revision 24
# baseline (speedup 1.0000x reference)
"""Trainium2 Bass kernel for quantized ConvBNReLU1D (pointwise conv k=1).

Reference computation (see problem spec):
    wq  = fake_quant_int8(W)  (per-tensor power-of-two scale)
    bq  = fake_quant_int8(b)
    y   = wq @ x + bq                  # [Cout,Cin] x [B,Cin,N]
    y   = y * inv + (beta - mean*inv)  # BN inference, inv = gamma*rsqrt(var+eps)
    y   = clip(round(relu(y)/as), 0, 255) * as   # QuantReLU

Strategy (v4 — minimize HBM bytes, keep PE/ACT/DVE/DMA all streaming):
  - Data-parallel over batch: 32 batches -> 4 per core on 8 cores.
  - Host precomputes per-channel constants (wq/bq fake-quant is bitwise
    the fp32 reference; BN + act-scale folded), packed into two DMAs.
  - x ships as a SINGLE fp16 copy: wq is exact in fp16 (8-bit int times
    a power of two), so the only error is fp16 rounding of x; measured
    absmax error vs the fp32 reference is 1 quant step (rel 0.0039,
    gate 2e-2).  Output leaves the device as u8 (the QuantReLU code);
    host reconstructs y = u8 * act_scale in fp32 (bit-identical to the
    same multiply on device).  12.6 MB HBM traffic per core.
  - Matmul: per (batch, mo) row block, k-outer sweep across all 8 PSUM
    banks (8x N=512 matmuls per weight chunk) so the stationary weight
    switches only twice per block: PE streams at the warm 216 ns/MM
    back-to-back rate.
  - Epilogue u8 = sat_u8(relu(psum*sv + bv)) alternates ScalarE
    activation / VectorE tensor_scalar per [128,1024] PSUM pair (both
    convert f32->u8 with exact RNE + [0,255] clamp, probe-verified
    bit-equal vs np.round incl. half-integers and negatives).
  - DMA rings: x loads split sync (k0) / scalar (k1); u8 stores on the
    gpsimd SWDGE ring in half-row chunks. Batch 0's x arrives as
    [128,1024] quarters so the PE starts ~3 us earlier.
"""

import os
import sys

import numpy as np

for _p in ("/opt/trn_rl_repo", "/root/.axon_site/_ro/trn_rl_repo"):
    if os.path.isdir(_p) and _p not in sys.path:
        sys.path.insert(0, _p)

from contextlib import ExitStack

import concourse.bacc as bacc
import concourse.tile as tile
from concourse import mybir
from concourse.bass import ts
from concourse.bass_utils import run_bass_kernel_spmd

F32 = mybir.dt.float32
F16 = mybir.dt.float16
U8 = mybir.dt.uint8
AF = mybir.ActivationFunctionType
ALU = mybir.AluOpType

N_CORES = 8
B, CIN, COUT, N = 32, 256, 256, 4096
B_SH = B // N_CORES  # batches per core
KC = CIN // 128      # K chunks
MC = COUT // 128     # output-channel chunks
NTILE = 512          # matmul free dim (one fp32 PSUM bank)
HW_ = 1024           # epilogue tile width (2 PSUM banks)
NH = N // HW_        # epilogue tiles per [128, N] row block

QMAX_W = 127.0
BN_EPS = 1e-5

_NC_CACHE = []
LAST_RESULTS = None  # BassKernelResults of the last run (for profiling)


def _build_nc():
    nc = bacc.Bacc("TRN2", target_bir_lowering=False)
    x_s = nc.declare_dram_parameter("x_s", [B_SH, CIN, N], F16, isOutput=False)
    # wcat[:, (2k+mo)*128:(2k+mo+1)*128] = wq.T chunk (k, mo)
    wcat = nc.declare_dram_parameter("wcat", [128, KC * MC * 128], F16, isOutput=False)
    # svbv cols: [sv_mo0, sv_mo1, bv_mo0, bv_mo1]
    svbv = nc.declare_dram_parameter("svbv", [128, 2 * MC], F32, isOutput=False)
    y_u8 = nc.declare_dram_parameter("y_u8", [B_SH, COUT, N], U8, isOutput=True)

    with ExitStack() as ctx:
        tc = ctx.enter_context(tile.TileContext(nc))
        consts = ctx.enter_context(tc.tile_pool(name="consts", bufs=1))
        # whole per-core x stays resident: loads never wait on recycling
        xqpool = ctx.enter_context(tc.tile_pool(name="xqpool", bufs=KC * 2))
        xpool = ctx.enter_context(tc.tile_pool(name="xpool", bufs=KC * (B_SH - 1)))
        opool = ctx.enter_context(tc.tile_pool(name="opool", bufs=5))
        pspool = ctx.enter_context(tc.tile_pool(name="pspool", bufs=4, space="PSUM"))

        # Packed constants on the scalar ring (stores come much later), so
        # the sync ring starts streaming x immediately.
        wt = consts.tile([128, KC * MC * 128], F16, tag="w")
        nc.scalar.dma_start(out=wt, in_=wcat[:, :])
        w_sb = {
            (k, mo): wt[:, ts(2 * k + mo, 128)] for k in range(KC) for mo in range(MC)
        }
        sb = consts.tile([128, 2 * MC], F32, tag="svbv")
        nc.scalar.dma_start(out=sb, in_=svbv[:, :])
        sv_sb = [sb[:, mo : mo + 1] for mo in range(MC)]
        bv_sb = [sb[:, MC + mo : MC + mo + 1] for mo in range(MC)]

        # Warm the ACT activation table + operand registers during the
        # x-load dead time, so the first real epilogue doesn't eat the
        # one-time ~1.3 us ACT_TABLE_LOAD mid-pipeline.
        wu_in = consts.tile([128, 8], F32, tag="wu_in")
        nc.vector.memset(wu_in, 0.0)
        wu_out = consts.tile([128, 8], U8, tag="wu_out")
        nc.scalar.activation(
            wu_out, wu_in, AF.Relu, bias=bv_sb[0], scale=sv_sb[0]
        )

        # x loads all on the sync ring (stores own the scalar ring).
        # Batch 0 arrives in [128,1024] quarters, interleaved k0/k1, so
        # the first matmul can start after ~0.5 MB instead of 2 MB.
        x_sb = {}  # (b, k) -> list of tiles covering [0, N)
        for k in range(KC):
            x_sb[(0, k)] = []
        for q in range(2):
            for k in range(KC):
                xt = xqpool.tile([128, N // 2], F16, tag=f"xq{k}", name=f"xq{k}_{q}")
                nc.sync.dma_start(
                    out=xt, in_=x_s[0, k * 128 : (k + 1) * 128, ts(q, N // 2)]
                )
                x_sb[(0, k)].append(xt)
        for b in range(1, B_SH):
            for k in range(KC):
                xt = xpool.tile([128, N], F16, tag=f"x{k}", name=f"x{k}_{b}")
                nc.sync.dma_start(out=xt, in_=x_s[b, k * 128 : (k + 1) * 128, :])
                x_sb[(b, k)] = [xt]

        def rhs(b, k, h, j):
            parts = x_sb[(b, k)]
            col = h * HW_ + j * NTILE
            pw = N // len(parts)
            return parts[col // pw][:, col % pw : col % pw + NTILE]

        ep = 0  # alternates epilogue tiles between ScalarE and VectorE
        for b in range(B_SH):
            for mo in range(MC):
                ot = opool.tile([128, N], U8, tag="o")
                for h in range(NH):
                    ps = pspool.tile([128, HW_], F32, tag="ps")
                    for k in range(KC):
                        for j in range(HW_ // NTILE):
                            nc.tensor.matmul(
                                ps[:, ts(j, NTILE)],
                                lhsT=w_sb[(k, mo)],
                                rhs=rhs(b, k, h, j),
                                start=(k == 0),
                                stop=(k == KC - 1),
                            )
                    # u8 = sat_u8(relu(psum*sv + bv)); f32->u8 is exact
                    # RNE + clamp on both engines (probe-verified).
                    if ep % 2 == 0:
                        nc.scalar.activation(
                            ot[:, ts(h, HW_)], ps, AF.Relu,
                            bias=bv_sb[mo], scale=sv_sb[mo],
                        )
                    else:
                        nc.vector.tensor_scalar(
                            ot[:, ts(h, HW_)], ps, sv_sb[mo],
                            bv_sb[mo], ALU.mult, ALU.add,
                        )
                    ep += 1
                    # Stores ride the SAME sync ring as the loads: HWDGE
                    # rings drain FIFO per ring, so no store byte moves
                    # until every load byte has — loads get 100% of SDMA
                    # bandwidth while the PE depends on them, then stores
                    # blast at full rate. Full-block stores (4 KB lines)
                    # except the final row block (halved to trim the tail).
                    last = b == B_SH - 1 and mo == MC - 1
                    if last and h == NH // 2 - 1:
                        nc.sync.dma_start(
                            out=y_u8[b, mo * 128 : (mo + 1) * 128, : N // 2],
                            in_=ot[:, : N // 2],
                        )
                    elif last and h == NH - 1:
                        nc.sync.dma_start(
                            out=y_u8[b, mo * 128 : (mo + 1) * 128, N // 2 :],
                            in_=ot[:, N // 2 :],
                        )
                    elif not last and h == NH - 1:
                        nc.sync.dma_start(
                            out=y_u8[b, mo * 128 : (mo + 1) * 128, :], in_=ot
                        )
    nc.compile()
    return nc


def _host_fold(W, b, gamma, beta, running_mean, running_var, act_scale):
    """Fake-quant W/b exactly as the fp32 reference, fold BN + act scale."""
    f32 = np.float32

    def po2_scale(t):
        maxabs = np.maximum(np.max(np.abs(t)), f32(1e-12)).astype(f32)
        # log2/ceil/exp2 of an f32 value; result is an exact power of two.
        return np.exp2(np.ceil(np.log2(maxabs / f32(QMAX_W)))).astype(f32)

    def fake_quant(t, s):
        return (np.clip(np.round(t / s), -128.0, 127.0) * s).astype(f32)

    wq = fake_quant(W.astype(f32), po2_scale(W.astype(f32)))
    bq = fake_quant(b.astype(f32), po2_scale(b.astype(f32)))
    inv = (gamma.astype(f32) / np.sqrt(running_var.astype(f32) + f32(BN_EPS))).astype(f32)
    shift = (beta.astype(f32) - running_mean.astype(f32) * inv).astype(f32)
    a_s = f32(act_scale)
    sv = (inv / a_s).astype(f32)                    # per-channel matmul scale
    bv = ((bq * inv + shift) / a_s).astype(f32)     # per-channel bias
    # wq is an 8-bit integer times a power of two -> exact in fp16
    wT = np.ascontiguousarray(wq.T).astype(np.float16)
    return wT, sv, bv, a_s


def kernel(x, W, b, gamma, beta, running_mean, running_var, act_scale):
    global LAST_RESULTS
    if not _NC_CACHE:
        _NC_CACHE.append(_build_nc())
    nc = _NC_CACHE[0]

    wT, sv, bv, a_s = _host_fold(
        W, b, gamma, beta, running_mean, running_var, act_scale
    )
    wcat = np.empty((128, KC * MC * 128), np.float16)
    for k in range(KC):
        for mo in range(MC):
            wcat[:, (2 * k + mo) * 128 : (2 * k + mo + 1) * 128] = wT[
                k * 128 : (k + 1) * 128, mo * 128 : (mo + 1) * 128
            ]
    svbv = np.empty((128, 2 * MC), np.float32)
    for mo in range(MC):
        svbv[:, mo] = sv[mo * 128 : (mo + 1) * 128]
        svbv[:, MC + mo] = bv[mo * 128 : (mo + 1) * 128]

    x_f16 = np.ascontiguousarray(np.asarray(x, dtype=np.float32)).astype(np.float16)

    in_maps = []
    for c in range(N_CORES):
        sl = slice(c * B_SH, (c + 1) * B_SH)
        in_maps.append({"x_s": x_f16[sl], "wcat": wcat, "svbv": svbv})

    trace = bool(os.environ.get("KERNEL_TRACE"))
    try:
        res = run_bass_kernel_spmd(
            nc, in_maps, core_ids=list(range(N_CORES)), trace=trace
        )
    except Exception:
        if not trace:
            raise
        # trace path unavailable (e.g. NTFF hook missing) — run untraced
        res = run_bass_kernel_spmd(
            nc, in_maps, core_ids=list(range(N_CORES)), trace=False
        )
    LAST_RESULTS = res
    u8 = np.concatenate([r["y_u8"] for r in res.results], axis=0)
    return u8.astype(np.float32) * a_s
